# revision 2
# baseline (speedup 1.0000x reference)
"""Trainium2 Bass kernel for a single AttnDecoderRNN step (batch=1), tensor-parallel
across 8 NeuronCores.

Computation (see reference): embedding lookup -> Bahdanau attention over 128
encoder positions -> attn_combine + relu -> one GRU step -> vocab projection
(50257) -> log_softmax. Returns (log_probs [1,V], h_new [1,1,H], attn_w [1,L]).

Distribution strategy (single uniform NEFF on cores 0..7; per-core DATA differs):
  - emb is hidden-sharded: core j holds emb[:, 128j:128j+128] and gathers its
    128-wide slice of the embedded token with an indirect DMA; an AllGather
    assembles the full q on every core.
  - attention (1.5MB of weights) is replicated: every core computes the full
    softmax attention row.
  - attn_combine is row-sharded (each core computes its 128 rows of `combined`,
    which is exactly its contraction slice of the GRU input), and the GRU mats
    are column-sharded over hidden; one AllReduce sums the [gi|gh] partials,
    after which every core computes h_new locally.
  - out_W is row-sharded over vocab (6656 rows/core, padded 50257->53248 with
    zero weights / -1e9 bias so padded logits contribute exp()==0); each core
    streams its 26.6MB shard through the PE as [1,512] matvec chunks,
    accumulating exp() sums on the fly. One tiny AllGather combines the
    softmax normalizers; each core writes its normalized log_prob chunk.

All weights are passed PRE-TRANSPOSED from the host ([in,out] layout) so every
DMA is naturally contiguous and the PE streams the moving operand at line rate;
activations that feed matmul stationary operands are passed (or computed)
in partition-major layout. log_softmax skips the max-subtraction: logits here
are ~N(0, 0.6), so exp() cannot overflow in fp32 and the math is identical.
"""

import os

import numpy as np

import concourse.bass as bass
import concourse.mybir as mybir
import concourse.tile as tile
from concourse import bacc
from concourse.bass_utils import run_bass_kernel_spmd
from concourse.masks import make_identity

N_CORES = 8
H = 1024
HC = H // N_CORES          # 128, per-core hidden slice
L = 128                    # encoder length
V = 50257
VC = 6656                  # vocab rows per core (13 x 512)
V_PAD = VC * N_CORES       # 53248
G3 = 3 * H                 # 3072 GRU gate rows
GC = G3 // 128             # 24 gate partition-major columns
NEG_BIG = -1.0e9

F32 = mybir.dt.float32
I32 = mybir.dt.int32
AF = mybir.ActivationFunctionType
ALU = mybir.AluOpType

_COMPILED = {}


def _build():
    nc = bacc.Bacc("TRN2", target_bir_lowering=False, debug=False,
                   num_devices=N_CORES)

    def din(name, shape, dt=F32):
        return nc.dram_tensor(name, shape, dt, kind="ExternalInput").ap()

    def dout(name, shape, dt=F32):
        return nc.dram_tensor(name, shape, dt, kind="ExternalOutput").ap()

    ids2 = din("ids2", [2, 1], I32)
    emb_sh = din("emb_sh", [V, HC])
    attn_WT = din("attn_WT", [2 * H, L])
    attn_b_row = din("attn_b_row", [1, L])
    enc = din("enc", [L, H])
    comb_WT_sh = din("comb_WT_sh", [2 * H, HC])
    comb_b_row = din("comb_b_row", [1, HC])
    W_ihT_sh = din("W_ihT_sh", [HC, G3])
    W_hhT_sh = din("W_hhT_sh", [HC, G3])
    h_chunk_pm = din("h_chunk_pm", [HC, 1])
    h_pm = din("h_pm", [128, H // 128])
    b_ih_pm = din("b_ih_pm", [128, GC])
    b_hh_pm = din("b_hh_pm", [128, GC])
    out_WT_sh = din("out_WT_sh", [H, VC])
    out_b_row = din("out_b_row", [1, VC])

    out_logp = dout("out_logp", [1, VC])
    out_h = dout("out_h", [H // 128, 128])
    out_attnw = dout("out_attnw", [1, L])

    KT = 2 * H // 128      # 16 contraction tiles for the 2048-dim matvecs
    KH = H // 128          # 8 contraction tiles for 1024-dim matvecs
    NVC = VC // 512        # 13 vocab chunks per core

    with tile.TileContext(nc) as tc:
        with tc.tile_pool(name="const", bufs=1) as constp, \
             tc.tile_pool(name="wts", bufs=1) as wts, \
             tc.tile_pool(name="act", bufs=1) as act, \
             tc.tile_pool(name="stream", bufs=3) as stream, \
             tc.tile_pool(name="scratch", bufs=2) as scratch, \
             tc.tile_pool(name="ps", bufs=3, space="PSUM") as ps, \
             tc.tile_pool(name="ps_log", bufs=4, space="PSUM") as ps_log, \
             tc.tile_pool(name="dram", bufs=1, space="DRAM") as dram:

            # ---- constants ----
            one1 = constp.tile([1, 1], F32)
            nc.vector.memset(one1[:], 1.0)
            ones8 = constp.tile([8, 1], F32)
            nc.vector.memset(ones8[:], 1.0)
            ident128 = constp.tile([128, 128], F32)
            make_identity(nc, ident128[:])

            # ---- weight loads (contiguous; small ones first) ----
            ids_sb = act.tile([2, 1], I32)
            nc.sync.dma_start(out=ids_sb[:], in_=ids2)
            attn_sb = wts.tile([128, KT, L], F32)
            nc.sync.dma_start(out=attn_sb[:],
                              in_=attn_WT.rearrange("(t k) n -> k t n", k=128))
            attn_b_sb = act.tile([1, L], F32)
            nc.sync.dma_start(out=attn_b_sb[:], in_=attn_b_row)
            enc_sb = wts.tile([L, H], F32)
            nc.sync.dma_start(out=enc_sb[:], in_=enc)
            comb_sb = wts.tile([128, KT, HC], F32)
            nc.sync.dma_start(out=comb_sb[:],
                              in_=comb_WT_sh.rearrange("(t k) n -> k t n", k=128))
            comb_b_sb = act.tile([1, HC], F32)
            nc.sync.dma_start(out=comb_b_sb[:], in_=comb_b_row)
            wih_sb = wts.tile([HC, G3], F32)
            nc.sync.dma_start(out=wih_sb[:], in_=W_ihT_sh)
            whh_sb = wts.tile([HC, G3], F32)
            nc.sync.dma_start(out=whh_sb[:], in_=W_hhT_sh)
            hck_sb = act.tile([HC, 1], F32)
            nc.sync.dma_start(out=hck_sb[:], in_=h_chunk_pm)
            hpm_sb = act.tile([128, KH], F32)
            nc.sync.dma_start(out=hpm_sb[:], in_=h_pm)
            bih_sb = act.tile([128, GC], F32)
            nc.sync.dma_start(out=bih_sb[:], in_=b_ih_pm)
            bhh_sb = act.tile([128, GC], F32)
            nc.sync.dma_start(out=bhh_sb[:], in_=b_hh_pm)
            outb_sb = act.tile([1, VC], F32)
            nc.sync.dma_start(out=outb_sb[:], in_=out_b_row)

            # ---- embedding gather + AllGather of q ----
            q2 = act.tile([2, HC], F32)
            nc.gpsimd.indirect_dma_start(
                out=q2[:], out_offset=None, in_=emb_sh,
                in_offset=bass.IndirectOffsetOnAxis(ap=ids_sb[:, :1], axis=0))
            q_bounce = dram.tile([1, HC], F32)
            nc.sync.dma_start(out=q_bounce[:], in_=q2[0:1, :])
            q_all = dram.tile([N_CORES, HC], F32, addr_space="Shared")
            nc.gpsimd.collective_compute(
                "AllGather", ALU.bypass,
                replica_groups=[list(range(N_CORES))],
                ins=[q_bounce.opt()], outs=[q_all.opt()])
            q_rows = act.tile([N_CORES, HC], F32)
            nc.sync.dma_start(out=q_rows[:], in_=q_all[:])

            # q partition-major [128, 8]: q_pm[k, c] = q[c*128 + k]
            q_pm_ps = ps.tile([128, KH], F32, tag="ps_small")
            nc.tensor.matmul(out=q_pm_ps[:], lhsT=q_rows[:],
                             rhs=ident128[:N_CORES, :N_CORES],
                             start=True, stop=True)
            q_pm = act.tile([128, KH], F32)
            nc.vector.tensor_copy(out=q_pm[:], in_=q_pm_ps[:])

            # ---- attention scores: [1, L] = cat(q, h) @ attn_W.T ----
            sc_ps = ps.tile([1, L], F32, tag="ps_small")
            for t in range(KT):
                lhs = q_pm[:, t:t + 1] if t < KH else hpm_sb[:, t - KH:t - KH + 1]
                nc.tensor.matmul(out=sc_ps[:], lhsT=lhs, rhs=attn_sb[:, t, :],
                                 start=(t == 0), stop=(t == KT - 1))
            scores = act.tile([1, L], F32)
            nc.vector.tensor_add(out=scores[:], in0=sc_ps[:], in1=attn_b_sb[:])

            # ---- softmax over [1, L] ----
            smax = act.tile([1, 1], F32)
            nc.vector.tensor_reduce(out=smax[:], in_=scores[:],
                                    axis=mybir.AxisListType.X, op=ALU.max)
            neg_m = act.tile([1, 1], F32)
            nc.vector.tensor_scalar_mul(neg_m[:], smax[:], -1.0)
            e_row = act.tile([1, L], F32)
            se = act.tile([1, 1], F32)
            nc.scalar.activation(out=e_row[:], in_=scores[:], func=AF.Exp,
                                 bias=neg_m[:, 0:1], scale=1.0, accum_out=se[:])
            rse = act.tile([1, 1], F32)
            nc.vector.reciprocal(out=rse[:], in_=se[:])
            attn_w = act.tile([1, L], F32)
            nc.vector.tensor_scalar_mul(attn_w[:], e_row[:], rse[:, 0:1])
            nc.sync.dma_start(out=out_attnw, in_=attn_w[:])

            # attn_w to partition-major [128, 1]
            aw_ps = ps.tile([L, 1], F32, tag="ps_small")
            nc.tensor.matmul(out=aw_ps[:], lhsT=attn_w[:], rhs=one1[:],
                             start=True, stop=True)
            aw_pm = act.tile([L, 1], F32)
            nc.vector.tensor_copy(out=aw_pm[:], in_=aw_ps[:])

            # ---- attn_applied, directly partition-major [128, 8] ----
            aa_ps = ps.tile([128, KH], F32, tag="ps_small")
            for c in range(KH):
                nc.tensor.matmul(out=aa_ps[:, c:c + 1],
                                 lhsT=enc_sb[:, c * 128:(c + 1) * 128],
                                 rhs=aw_pm[:], start=True, stop=True)
            aa_pm = act.tile([128, KH], F32)
            nc.vector.tensor_copy(out=aa_pm[:], in_=aa_ps[:])

            # ---- combined chunk (this core's 128 rows) + relu -> gru_in ----
            cb_ps = ps.tile([1, HC], F32, tag="ps_small")
            for t in range(KT):
                lhs = q_pm[:, t:t + 1] if t < KH else aa_pm[:, t - KH:t - KH + 1]
                nc.tensor.matmul(out=cb_ps[:], lhsT=lhs, rhs=comb_sb[:, t, :],
                                 start=(t == 0), stop=(t == KT - 1))
            gru_row = act.tile([1, HC], F32)
            nc.vector.tensor_add(out=gru_row[:], in0=cb_ps[:], in1=comb_b_sb[:])
            nc.vector.tensor_scalar_max(gru_row[:], gru_row[:], 0.0)

            gin_ps = ps.tile([HC, 1], F32, tag="ps_small")
            nc.tensor.matmul(out=gin_ps[:], lhsT=gru_row[:], rhs=one1[:],
                             start=True, stop=True)
            gin_pm = act.tile([HC, 1], F32)
            nc.vector.tensor_copy(out=gin_pm[:], in_=gin_ps[:])

            # ---- GRU gate partials: [128, 48] = [gi(24) | gh(24)] ----
            g_ps = ps.tile([128, 2 * GC], F32, tag="ps_small")
            for c in range(GC):
                nc.tensor.matmul(out=g_ps[:, c:c + 1],
                                 lhsT=wih_sb[:, c * 128:(c + 1) * 128],
                                 rhs=gin_pm[:], start=True, stop=True)
            for c in range(GC):
                nc.tensor.matmul(out=g_ps[:, GC + c:GC + c + 1],
                                 lhsT=whh_sb[:, c * 128:(c + 1) * 128],
                                 rhs=hck_sb[:], start=True, stop=True)
            g_sb = act.tile([128, 2 * GC], F32)
            nc.vector.tensor_copy(out=g_sb[:], in_=g_ps[:])

            g_bounce = dram.tile([128, 2 * GC], F32)
            nc.sync.dma_start(out=g_bounce[:], in_=g_sb[:])
            g_red = dram.tile([128, 2 * GC], F32, addr_space="Shared")
            nc.gpsimd.collective_compute(
                "AllReduce", ALU.add,
                replica_groups=[list(range(N_CORES))],
                ins=[g_bounce.opt()], outs=[g_red.opt()])
            gs = act.tile([128, 2 * GC], F32)
            nc.sync.dma_start(out=gs[:], in_=g_red[:])

            # ---- gates -> h_new (partition-major [128, 8]) ----
            gi = act.tile([128, GC], F32)
            nc.vector.tensor_add(out=gi[:], in0=gs[:, 0:GC], in1=bih_sb[:])
            gh = act.tile([128, GC], F32)
            nc.vector.tensor_add(out=gh[:], in0=gs[:, GC:2 * GC], in1=bhh_sb[:])
            KZ = KH  # 8 columns per gate
            r_pre = act.tile([128, KZ], F32)
            nc.vector.tensor_add(out=r_pre[:], in0=gi[:, 0:KZ], in1=gh[:, 0:KZ])
            r_g = act.tile([128, KZ], F32)
            nc.scalar.activation(out=r_g[:], in_=r_pre[:], func=AF.Sigmoid)
            z_pre = act.tile([128, KZ], F32)
            nc.vector.tensor_add(out=z_pre[:], in0=gi[:, KZ:2 * KZ],
                                 in1=gh[:, KZ:2 * KZ])
            z_g = act.tile([128, KZ], F32)
            nc.scalar.activation(out=z_g[:], in_=z_pre[:], func=AF.Sigmoid)
            n_pre = act.tile([128, KZ], F32)
            nc.vector.tensor_mul(out=n_pre[:], in0=r_g[:], in1=gh[:, 2 * KZ:3 * KZ])
            nc.vector.tensor_add(out=n_pre[:], in0=n_pre[:], in1=gi[:, 2 * KZ:3 * KZ])
            n_g = act.tile([128, KZ], F32)
            nc.scalar.activation(out=n_g[:], in_=n_pre[:], func=AF.Tanh)
            # h_new = n + z * (h - n)
            hmn = act.tile([128, KZ], F32)
            nc.vector.tensor_sub(out=hmn[:], in0=hpm_sb[:], in1=n_g[:])
            nc.vector.tensor_mul(out=hmn[:], in0=hmn[:], in1=z_g[:])
            hn_pm = act.tile([128, KZ], F32)
            nc.vector.tensor_add(out=hn_pm[:], in0=n_g[:], in1=hmn[:])

            # h_new rows for output: [8, 128]
            hr_ps = ps.tile([KH, 128], F32, tag="ps_small")
            nc.tensor.matmul(out=hr_ps[:], lhsT=hn_pm[:], rhs=ident128[:],
                             start=True, stop=True)
            hn_rows = act.tile([KH, 128], F32)
            nc.vector.tensor_copy(out=hn_rows[:], in_=hr_ps[:])
            nc.sync.dma_start(out=out_h, in_=hn_rows[:])

            # ---- vocab projection: stream out_WT, 13 chunks of [1, 512] ----
            logits = act.tile([1, VC], F32)
            se_acc = act.tile([1, NVC], F32)
            wt_r = out_WT_sh.rearrange("(kk p) n -> p kk n", p=128)
            for vc in range(NVC):
                st = stream.tile([128, KH, 512], F32, tag="owt")
                nc.sync.dma_start(out=st[:],
                                  in_=wt_r[:, :, vc * 512:(vc + 1) * 512])
                lp = ps_log.tile([1, 512], F32, tag="lps")
                for k in range(KH):
                    nc.tensor.matmul(out=lp[:], lhsT=hn_pm[:, k:k + 1],
                                     rhs=st[:, k, :],
                                     start=(k == 0), stop=(k == KH - 1))
                nc.vector.tensor_add(out=logits[:, vc * 512:(vc + 1) * 512],
                                     in0=lp[:], in1=outb_sb[:, vc * 512:(vc + 1) * 512])
                er = scratch.tile([1, 512], F32, tag="er")
                nc.scalar.activation(out=er[:],
                                     in_=logits[:, vc * 512:(vc + 1) * 512],
                                     func=AF.Exp, bias=0.0, scale=1.0,
                                     accum_out=se_acc[:, vc:vc + 1])

            s_loc = act.tile([1, 1], F32)
            nc.vector.tensor_reduce(out=s_loc[:], in_=se_acc[:],
                                    axis=mybir.AxisListType.X, op=ALU.add)
            st_row = act.tile([1, 8], F32)
            nc.vector.memset(st_row[:], 0.0)
            nc.vector.tensor_copy(out=st_row[:, 0:1], in_=s_loc[:])
            st_bounce = dram.tile([1, 8], F32)
            nc.sync.dma_start(out=st_bounce[:], in_=st_row[:])
            st_all = dram.tile([N_CORES, 8], F32, addr_space="Shared")
            nc.gpsimd.collective_compute(
                "AllGather", ALU.bypass,
                replica_groups=[list(range(N_CORES))],
                ins=[st_bounce.opt()], outs=[st_all.opt()])
            st_sb = act.tile([N_CORES, 8], F32)
            nc.sync.dma_start(out=st_sb[:], in_=st_all[:])
            sg_ps = ps.tile([1, 1], F32, tag="ps_small")
            nc.tensor.matmul(out=sg_ps[:], lhsT=st_sb[:, 0:1], rhs=ones8[:],
                             start=True, stop=True)
            logz = act.tile([1, 1], F32)
            nc.scalar.activation(out=logz[:], in_=sg_ps[:], func=AF.Ln)
            neg_lz = act.tile([1, 1], F32)
            nc.vector.tensor_scalar_mul(neg_lz[:], logz[:], -1.0)

            nc.vector.tensor_scalar_add(logits[:], logits[:], neg_lz[:, 0:1])
            nc.sync.dma_start(out=out_logp, in_=logits[:])

    nc.compile()
    return nc


def _get_nc():
    if "nc" not in _COMPILED:
        _COMPILED["nc"] = _build()
    return _COMPILED["nc"]


def _ct(x):
    return np.ascontiguousarray(x, dtype=np.float32)


def kernel(input_ids, hidden, encoder_outputs, emb, attn_W, attn_b,
           comb_W, comb_b, W_ih, W_hh, b_ih, b_hh, out_W, out_b):
    input_ids = np.asarray(input_ids)
    hidden = np.asarray(hidden, dtype=np.float32)
    encoder_outputs = np.asarray(encoder_outputs, dtype=np.float32)
    emb = np.asarray(emb, dtype=np.float32)
    attn_W = np.asarray(attn_W, dtype=np.float32)
    attn_b = np.asarray(attn_b, dtype=np.float32)
    comb_W = np.asarray(comb_W, dtype=np.float32)
    comb_b = np.asarray(comb_b, dtype=np.float32)
    W_ih = np.asarray(W_ih, dtype=np.float32)
    W_hh = np.asarray(W_hh, dtype=np.float32)
    b_ih = np.asarray(b_ih, dtype=np.float32)
    b_hh = np.asarray(b_hh, dtype=np.float32)
    out_W = np.asarray(out_W, dtype=np.float32)
    out_b = np.asarray(out_b, dtype=np.float32)

    nc = _get_nc()

    idx = int(np.asarray(input_ids).reshape(-1)[0])
    h = hidden.reshape(H)

    out_W_pad = np.zeros((V_PAD, H), np.float32)
    out_W_pad[:V] = out_W
    out_b_pad = np.full((V_PAD,), NEG_BIG, np.float32)
    out_b_pad[:V] = out_b

    attn_WT = _ct(attn_W.T)                      # [2048, 128]
    h_pm = _ct(h.reshape(H // 128, 128).T)       # [128, 8]
    b_ih_pm = _ct(b_ih.reshape(GC, 128).T)       # [128, 24]
    b_hh_pm = _ct(b_hh.reshape(GC, 128).T)
    ids2 = np.full((2, 1), idx, np.int32)
    attn_b_row = attn_b.reshape(1, L)
    enc_c = _ct(encoder_outputs)

    in_maps = []
    for j in range(N_CORES):
        hs = slice(j * HC, (j + 1) * HC)
        vs = slice(j * VC, (j + 1) * VC)
        in_maps.append({
            "ids2": ids2,
            "emb_sh": _ct(emb[:, hs]),
            "attn_WT": attn_WT,
            "attn_b_row": attn_b_row,
            "enc": enc_c,
            "comb_WT_sh": _ct(comb_W[hs, :].T),     # [2048, 128]
            "comb_b_row": _ct(comb_b[hs].reshape(1, HC)),
            "W_ihT_sh": _ct(W_ih[:, hs].T),         # [128, 3072]
            "W_hhT_sh": _ct(W_hh[:, hs].T),
            "h_chunk_pm": _ct(h[hs].reshape(HC, 1)),
            "h_pm": h_pm,
            "b_ih_pm": b_ih_pm,
            "b_hh_pm": b_hh_pm,
            "out_WT_sh": _ct(out_W_pad[vs, :].T),   # [1024, 6656]
            "out_b_row": _ct(out_b_pad[vs].reshape(1, VC)),
        })

    trace = bool(int(os.environ.get("KERNEL_TRACE", "0")))
    res = run_bass_kernel_spmd(nc, in_maps, core_ids=list(range(N_CORES)),
                               trace=trace)
    kernel.last_result = res

    logp = np.concatenate([res.results[j]["out_logp"][0] for j in range(N_CORES)])
    log_probs = logp[:V][None, :]
    h_new = res.results[0]["out_h"].reshape(1, 1, H)
    attn_weights = res.results[0]["out_attnw"].reshape(1, L)
    return log_probs, h_new, attn_weights


# revision 4
# speedup vs baseline: 1.4322x; 1.4322x over previous
"""Trainium2 Bass kernel for a single AttnDecoderRNN step (batch=1), tensor-parallel
across 8 NeuronCores.

Computation (see reference): embedding lookup -> Bahdanau attention over 128
encoder positions -> attn_combine + relu -> one GRU step -> vocab projection
(50257) -> log_softmax. Returns (log_probs [1,V], h_new [1,1,H], attn_w [1,L]).

Distribution strategy (single uniform NEFF on cores 0..7; per-core DATA differs):
  - emb is hidden-sharded: core j holds emb[:, 128j:128j+128] and gathers its
    128-wide slice of the embedded token with an indirect DMA; an AllGather
    assembles the full q on every core.
  - attention (1.5MB of weights) is replicated: every core computes the full
    softmax attention row.
  - attn_combine is row-sharded (each core computes its 128 rows of `combined`,
    which is exactly its contraction slice of the GRU input), and the GRU mats
    are column-sharded over hidden; one AllReduce sums the [gi|gh] partials,
    after which every core computes h_new locally.
  - out_W is row-sharded over vocab (6656 rows/core, padded 50257->53248 with
    zero weights / -1e9 bias so padded logits contribute exp()==0); each core
    streams its 26.6MB shard through the PE as [1,512] matvec chunks,
    accumulating exp() sums on the fly. One tiny AllGather combines the
    softmax normalizers; each core writes its normalized log_prob chunk.

All weights are passed PRE-TRANSPOSED from the host ([in,out] layout) so every
DMA is naturally contiguous and the PE streams the moving operand at line rate;
activations that feed matmul stationary operands are passed (or computed)
in partition-major layout. log_softmax skips the max-subtraction: logits here
are ~N(0, 0.6), so exp() cannot overflow in fp32 and the math is identical.
"""

import os

import ml_dtypes
import numpy as np

import concourse.bass as bass
import concourse.mybir as mybir
import concourse.tile as tile
from concourse import bacc
from concourse.bass_utils import run_bass_kernel_spmd
from concourse.masks import make_identity

N_CORES = 8
H = 1024
HC = H // N_CORES          # 128, per-core hidden slice
L = 128                    # encoder length
V = 50257
VC = 6656                  # vocab rows per core (13 x 512)
V_PAD = VC * N_CORES       # 53248
G3 = 3 * H                 # 3072 GRU gate rows
GC = G3 // 128             # 24 gate partition-major columns
NEG_BIG = -1.0e9

F32 = mybir.dt.float32
BF16 = mybir.dt.bfloat16
I32 = mybir.dt.int32
AF = mybir.ActivationFunctionType
ALU = mybir.AluOpType

_COMPILED = {}


def _build():
    nc = bacc.Bacc("TRN2", target_bir_lowering=False, debug=False,
                   num_devices=N_CORES)

    def din(name, shape, dt=F32):
        return nc.dram_tensor(name, shape, dt, kind="ExternalInput").ap()

    def dout(name, shape, dt=F32):
        return nc.dram_tensor(name, shape, dt, kind="ExternalOutput").ap()

    ids2 = din("ids2", [2, 1], I32)
    emb_sh = din("emb_sh", [V, HC])
    attn_WT = din("attn_WT", [2 * H, L])
    attn_b_row = din("attn_b_row", [1, L])
    enc = din("enc", [L, H])
    comb_WT_sh = din("comb_WT_sh", [2 * H, HC])
    comb_b_row = din("comb_b_row", [1, HC])
    W_ihT_sh = din("W_ihT_sh", [H, 3 * HC])
    W_hhT_sh = din("W_hhT_sh", [H, 3 * HC])
    h_pm = din("h_pm", [128, H // 128])
    h_chunk_row = din("h_chunk_row", [1, HC])
    b_ih_row = din("b_ih_row", [1, 3 * HC])
    b_hh_row = din("b_hh_row", [1, 3 * HC])
    out_WT_sh = din("out_WT_sh", [H, VC], BF16)
    out_b_row = din("out_b_row", [1, VC])

    out_logp = dout("out_logp", [1, VC])
    out_h = dout("out_h", [H // 128, 128])
    out_attnw = dout("out_attnw", [1, L])

    KT = 2 * H // 128      # 16 contraction tiles for the 2048-dim matvecs
    KH = H // 128          # 8 contraction tiles for 1024-dim matvecs
    NVC = VC // 512        # 13 vocab chunks per core

    with tile.TileContext(nc) as tc:
        with tc.tile_pool(name="const", bufs=1) as constp, \
             tc.tile_pool(name="wts", bufs=1) as wts, \
             tc.tile_pool(name="act", bufs=1) as act, \
             tc.tile_pool(name="stream", bufs=9) as stream, \
             tc.tile_pool(name="scratch", bufs=2) as scratch, \
             tc.tile_pool(name="ps", bufs=3, space="PSUM") as ps, \
             tc.tile_pool(name="ps_log", bufs=4, space="PSUM") as ps_log, \
             tc.tile_pool(name="dram", bufs=1, space="DRAM") as dram:

            # ---- constants ----
            one1 = constp.tile([1, 1], F32)
            nc.vector.memset(one1[:], 1.0)
            ones8 = constp.tile([8, 1], F32)
            nc.vector.memset(ones8[:], 1.0)
            ident128 = constp.tile([128, 128], F32)
            make_identity(nc, ident128[:])

            # ---- weight loads (contiguous; small ones first) ----
            ids_sb = act.tile([2, 1], I32)
            nc.sync.dma_start(out=ids_sb[:], in_=ids2)
            attn_sb = wts.tile([128, KT, L], F32)
            nc.sync.dma_start(out=attn_sb[:],
                              in_=attn_WT.rearrange("(t k) n -> k t n", k=128))
            attn_b_sb = act.tile([1, L], F32)
            nc.sync.dma_start(out=attn_b_sb[:], in_=attn_b_row)
            enc_sb = wts.tile([L, H], F32)
            nc.sync.dma_start(out=enc_sb[:], in_=enc)
            comb_sb = wts.tile([128, KT, HC], F32)
            nc.sync.dma_start(out=comb_sb[:],
                              in_=comb_WT_sh.rearrange("(t k) n -> k t n", k=128))
            comb_b_sb = act.tile([1, HC], F32)
            nc.sync.dma_start(out=comb_b_sb[:], in_=comb_b_row)
            wih_sb = wts.tile([128, KH, 3 * HC], F32)
            nc.sync.dma_start(out=wih_sb[:],
                              in_=W_ihT_sh.rearrange("(t k) n -> k t n", k=128))
            whh_sb = wts.tile([128, KH, 3 * HC], F32)
            nc.sync.dma_start(out=whh_sb[:],
                              in_=W_hhT_sh.rearrange("(t k) n -> k t n", k=128))
            hpm_sb = act.tile([128, KH], F32)
            nc.sync.dma_start(out=hpm_sb[:], in_=h_pm)
            hrow_sb = act.tile([1, HC], F32)
            nc.sync.dma_start(out=hrow_sb[:], in_=h_chunk_row)
            bih_sb = act.tile([1, 3 * HC], F32)
            nc.sync.dma_start(out=bih_sb[:], in_=b_ih_row)
            bhh_sb = act.tile([1, 3 * HC], F32)
            nc.sync.dma_start(out=bhh_sb[:], in_=b_hh_row)
            outb_sb = act.tile([1, VC], F32)
            nc.sync.dma_start(out=outb_sb[:], in_=out_b_row)

            # ---- gh gate chunk (depends only on h; runs during the entry barrier) ----
            gh_ps = ps.tile([1, 3 * HC], F32, tag="ps_small")
            for t in range(KH):
                nc.tensor.matmul(out=gh_ps[:], lhsT=hpm_sb[:, t:t + 1],
                                 rhs=whh_sb[:, t, :],
                                 start=(t == 0), stop=(t == KH - 1))
            gh_row = act.tile([1, 3 * HC], F32)
            nc.vector.tensor_add(out=gh_row[:], in0=gh_ps[:], in1=bhh_sb[:])

            # ---- embedding gather + AllGather of q ----
            q2 = act.tile([2, HC], F32)
            nc.gpsimd.indirect_dma_start(
                out=q2[:], out_offset=None, in_=emb_sh,
                in_offset=bass.IndirectOffsetOnAxis(ap=ids_sb[:, :1], axis=0))
            q_bounce = dram.tile([1, HC], F32)
            nc.sync.dma_start(out=q_bounce[:], in_=q2[0:1, :])
            q_all = dram.tile([N_CORES, HC], F32, addr_space="Shared")
            nc.gpsimd.collective_compute(
                "AllGather", ALU.bypass,
                replica_groups=[list(range(N_CORES))],
                ins=[q_bounce.opt()], outs=[q_all.opt()])
            q_rows = act.tile([N_CORES, HC], F32)
            nc.sync.dma_start(out=q_rows[:], in_=q_all[:])

            # q partition-major [128, 8]: q_pm[k, c] = q[c*128 + k]
            q_pm_ps = ps.tile([128, KH], F32, tag="ps_small")
            nc.tensor.matmul(out=q_pm_ps[:], lhsT=q_rows[:],
                             rhs=ident128[:N_CORES, :N_CORES],
                             start=True, stop=True)
            q_pm = act.tile([128, KH], F32)
            nc.vector.tensor_copy(out=q_pm[:], in_=q_pm_ps[:])

            # ---- attention scores: [1, L] = cat(q, h) @ attn_W.T ----
            sc_ps = ps.tile([1, L], F32, tag="ps_small")
            for t in range(KT):
                lhs = q_pm[:, t:t + 1] if t < KH else hpm_sb[:, t - KH:t - KH + 1]
                nc.tensor.matmul(out=sc_ps[:], lhsT=lhs, rhs=attn_sb[:, t, :],
                                 start=(t == 0), stop=(t == KT - 1))
            scores = act.tile([1, L], F32)
            nc.vector.tensor_add(out=scores[:], in0=sc_ps[:], in1=attn_b_sb[:])

            # ---- softmax over [1, L] ----
            smax = act.tile([1, 1], F32)
            nc.vector.tensor_reduce(out=smax[:], in_=scores[:],
                                    axis=mybir.AxisListType.X, op=ALU.max)
            neg_m = act.tile([1, 1], F32)
            nc.vector.tensor_scalar_mul(neg_m[:], smax[:], -1.0)
            e_row = act.tile([1, L], F32)
            se = act.tile([1, 1], F32)
            nc.scalar.activation(out=e_row[:], in_=scores[:], func=AF.Exp,
                                 bias=neg_m[:, 0:1], scale=1.0, accum_out=se[:])
            rse = act.tile([1, 1], F32)
            nc.vector.reciprocal(out=rse[:], in_=se[:])
            attn_w = act.tile([1, L], F32)
            nc.vector.tensor_scalar_mul(attn_w[:], e_row[:], rse[:, 0:1])
            nc.sync.dma_start(out=out_attnw, in_=attn_w[:])

            # attn_w to partition-major [128, 1]
            aw_ps = ps.tile([L, 1], F32, tag="ps_small")
            nc.tensor.matmul(out=aw_ps[:], lhsT=attn_w[:], rhs=one1[:],
                             start=True, stop=True)
            aw_pm = act.tile([L, 1], F32)
            nc.vector.tensor_copy(out=aw_pm[:], in_=aw_ps[:])

            # ---- attn_applied, directly partition-major [128, 8] ----
            aa_ps = ps.tile([128, KH], F32, tag="ps_small")
            for c in range(KH):
                nc.tensor.matmul(out=aa_ps[:, c:c + 1],
                                 lhsT=enc_sb[:, c * 128:(c + 1) * 128],
                                 rhs=aw_pm[:], start=True, stop=True)
            aa_pm = act.tile([128, KH], F32)
            nc.vector.tensor_copy(out=aa_pm[:], in_=aa_ps[:])

            # ---- combined chunk (this core's 128 rows) + relu -> gru_in ----
            cb_ps = ps.tile([1, HC], F32, tag="ps_small")
            for t in range(KT):
                lhs = q_pm[:, t:t + 1] if t < KH else aa_pm[:, t - KH:t - KH + 1]
                nc.tensor.matmul(out=cb_ps[:], lhsT=lhs, rhs=comb_sb[:, t, :],
                                 start=(t == 0), stop=(t == KT - 1))
            gru_row = act.tile([1, HC], F32)
            nc.vector.tensor_add(out=gru_row[:], in0=cb_ps[:], in1=comb_b_sb[:])
            nc.vector.tensor_scalar_max(gru_row[:], gru_row[:], 0.0)

            # AllGather gru_in chunks -> full gru_in rows [8, 128]
            gin_bounce = dram.tile([1, HC], F32)
            nc.sync.dma_start(out=gin_bounce[:], in_=gru_row[:])
            gin_all = dram.tile([N_CORES, HC], F32, addr_space="Shared")
            nc.gpsimd.collective_compute(
                "AllGather", ALU.bypass,
                replica_groups=[list(range(N_CORES))],
                ins=[gin_bounce.opt()], outs=[gin_all.opt()])
            gin_rows = act.tile([N_CORES, HC], F32)
            nc.sync.dma_start(out=gin_rows[:], in_=gin_all[:])
            gin_pm_ps = ps.tile([128, KH], F32, tag="ps_small")
            nc.tensor.matmul(out=gin_pm_ps[:], lhsT=gin_rows[:],
                             rhs=ident128[:N_CORES, :N_CORES],
                             start=True, stop=True)
            gin_pm = act.tile([128, KH], F32)
            nc.vector.tensor_copy(out=gin_pm[:], in_=gin_pm_ps[:])

            # gi gate chunk [1, 384]
            gi_ps = ps.tile([1, 3 * HC], F32, tag="ps_small")
            for t in range(KH):
                nc.tensor.matmul(out=gi_ps[:], lhsT=gin_pm[:, t:t + 1],
                                 rhs=wih_sb[:, t, :],
                                 start=(t == 0), stop=(t == KH - 1))
            gi_row = act.tile([1, 3 * HC], F32)
            nc.vector.tensor_add(out=gi_row[:], in0=gi_ps[:], in1=bih_sb[:])

            # ---- gates -> h_new chunk [1, 128] (torch order r, z, n) ----
            r_pre = act.tile([1, HC], F32)
            nc.vector.tensor_add(out=r_pre[:], in0=gi_row[:, 0:HC],
                                 in1=gh_row[:, 0:HC])
            r_g = act.tile([1, HC], F32)
            nc.scalar.activation(out=r_g[:], in_=r_pre[:], func=AF.Sigmoid)
            z_pre = act.tile([1, HC], F32)
            nc.vector.tensor_add(out=z_pre[:], in0=gi_row[:, HC:2 * HC],
                                 in1=gh_row[:, HC:2 * HC])
            z_g = act.tile([1, HC], F32)
            nc.scalar.activation(out=z_g[:], in_=z_pre[:], func=AF.Sigmoid)
            n_pre = act.tile([1, HC], F32)
            nc.vector.tensor_mul(out=n_pre[:], in0=r_g[:], in1=gh_row[:, 2 * HC:3 * HC])
            nc.vector.tensor_add(out=n_pre[:], in0=n_pre[:], in1=gi_row[:, 2 * HC:3 * HC])
            n_g = act.tile([1, HC], F32)
            nc.scalar.activation(out=n_g[:], in_=n_pre[:], func=AF.Tanh)
            # h_new = n + z * (h - n)
            hmn = act.tile([1, HC], F32)
            nc.vector.tensor_sub(out=hmn[:], in0=hrow_sb[:], in1=n_g[:])
            nc.vector.tensor_mul(out=hmn[:], in0=hmn[:], in1=z_g[:])
            hn_ch = act.tile([1, HC], F32)
            nc.vector.tensor_add(out=hn_ch[:], in0=n_g[:], in1=hmn[:])

            # AllGather h_new chunks -> [8, 128] rows (= output layout)
            hn_bounce = dram.tile([1, HC], F32)
            nc.sync.dma_start(out=hn_bounce[:], in_=hn_ch[:])
            hn_all = dram.tile([N_CORES, HC], F32, addr_space="Shared")
            nc.gpsimd.collective_compute(
                "AllGather", ALU.bypass,
                replica_groups=[list(range(N_CORES))],
                ins=[hn_bounce.opt()], outs=[hn_all.opt()])
            # h_new rows for output: [8, 128]
            hn_rows = act.tile([KH, 128], F32)
            nc.sync.dma_start(out=hn_rows[:], in_=hn_all[:])
            nc.sync.dma_start(out=out_h, in_=hn_rows[:])
            hn_pm_ps = ps.tile([128, KH], F32, tag="ps_small")
            nc.tensor.matmul(out=hn_pm_ps[:], lhsT=hn_rows[:],
                             rhs=ident128[:N_CORES, :N_CORES],
                             start=True, stop=True)
            hn_bf = act.tile([128, KH], BF16)
            nc.vector.tensor_copy(out=hn_bf[:], in_=hn_pm_ps[:])

            # ---- vocab projection: stream out_WT, 13 chunks of [1, 512] ----
            logits = act.tile([1, VC], F32)
            se_acc = act.tile([1, NVC], F32)
            wt_r = out_WT_sh.rearrange("(kk p) n -> p kk n", p=128)
            for vc in range(NVC):
                st = stream.tile([128, KH, 512], BF16, tag="owt")
                nc.sync.dma_start(out=st[:],
                                  in_=wt_r[:, :, vc * 512:(vc + 1) * 512])
                lp = ps_log.tile([1, 512], F32, tag="lps")
                for k in range(KH):
                    nc.tensor.matmul(out=lp[:], lhsT=hn_bf[:, k:k + 1],
                                     rhs=st[:, k, :],
                                     start=(k == 0), stop=(k == KH - 1))
                nc.vector.tensor_add(out=logits[:, vc * 512:(vc + 1) * 512],
                                     in0=lp[:], in1=outb_sb[:, vc * 512:(vc + 1) * 512])
                er = scratch.tile([1, 512], F32, tag="er")
                nc.scalar.activation(out=er[:],
                                     in_=logits[:, vc * 512:(vc + 1) * 512],
                                     func=AF.Exp, bias=0.0, scale=1.0,
                                     accum_out=se_acc[:, vc:vc + 1])

            s_loc = act.tile([1, 1], F32)
            nc.vector.tensor_reduce(out=s_loc[:], in_=se_acc[:],
                                    axis=mybir.AxisListType.X, op=ALU.add)
            st_row = act.tile([1, 8], F32)
            nc.vector.memset(st_row[:], 0.0)
            nc.vector.tensor_copy(out=st_row[:, 0:1], in_=s_loc[:])
            st_bounce = dram.tile([1, 8], F32)
            nc.sync.dma_start(out=st_bounce[:], in_=st_row[:])
            st_all = dram.tile([N_CORES, 8], F32, addr_space="Shared")
            nc.gpsimd.collective_compute(
                "AllGather", ALU.bypass,
                replica_groups=[list(range(N_CORES))],
                ins=[st_bounce.opt()], outs=[st_all.opt()])
            st_sb = act.tile([N_CORES, 8], F32)
            nc.sync.dma_start(out=st_sb[:], in_=st_all[:])
            sg_ps = ps.tile([1, 1], F32, tag="ps_small")
            nc.tensor.matmul(out=sg_ps[:], lhsT=st_sb[:, 0:1], rhs=ones8[:],
                             start=True, stop=True)
            logz = act.tile([1, 1], F32)
            nc.scalar.activation(out=logz[:], in_=sg_ps[:], func=AF.Ln)
            neg_lz = act.tile([1, 1], F32)
            nc.vector.tensor_scalar_mul(neg_lz[:], logz[:], -1.0)

            halfv = VC // 2
            nc.vector.tensor_scalar_add(logits[:, 0:halfv], logits[:, 0:halfv],
                                        neg_lz[:, 0:1])
            nc.scalar.activation(out=logits[:, halfv:VC], in_=logits[:, halfv:VC],
                                 func=AF.Identity, bias=neg_lz[:, 0:1], scale=1.0)
            nc.sync.dma_start(out=out_logp, in_=logits[:])

    nc.compile()
    return nc


def _get_nc():
    if "nc" not in _COMPILED:
        _COMPILED["nc"] = _build()
    return _COMPILED["nc"]


def _ct(x):
    return np.ascontiguousarray(x, dtype=np.float32)


def kernel(input_ids, hidden, encoder_outputs, emb, attn_W, attn_b,
           comb_W, comb_b, W_ih, W_hh, b_ih, b_hh, out_W, out_b):
    input_ids = np.asarray(input_ids)
    hidden = np.asarray(hidden, dtype=np.float32)
    encoder_outputs = np.asarray(encoder_outputs, dtype=np.float32)
    emb = np.asarray(emb, dtype=np.float32)
    attn_W = np.asarray(attn_W, dtype=np.float32)
    attn_b = np.asarray(attn_b, dtype=np.float32)
    comb_W = np.asarray(comb_W, dtype=np.float32)
    comb_b = np.asarray(comb_b, dtype=np.float32)
    W_ih = np.asarray(W_ih, dtype=np.float32)
    W_hh = np.asarray(W_hh, dtype=np.float32)
    b_ih = np.asarray(b_ih, dtype=np.float32)
    b_hh = np.asarray(b_hh, dtype=np.float32)
    out_W = np.asarray(out_W, dtype=np.float32)
    out_b = np.asarray(out_b, dtype=np.float32)

    nc = _get_nc()

    idx = int(np.asarray(input_ids).reshape(-1)[0])
    h = hidden.reshape(H)

    out_W_pad = np.zeros((V_PAD, H), np.float32)
    out_W_pad[:V] = out_W
    out_b_pad = np.full((V_PAD,), NEG_BIG, np.float32)
    out_b_pad[:V] = out_b

    attn_WT = _ct(attn_W.T)                      # [2048, 128]
    h_pm = _ct(h.reshape(H // 128, 128).T)       # [128, 8]
    ids2 = np.full((2, 1), idx, np.int32)
    attn_b_row = attn_b.reshape(1, L)
    enc_c = _ct(encoder_outputs)

    def gate_rows(W, j):
        # hidden-aligned row triple (r, z, n chunks j) of a [3H, x] gate matrix
        return np.concatenate([W[j * HC:(j + 1) * HC],
                               W[H + j * HC:H + (j + 1) * HC],
                               W[2 * H + j * HC:2 * H + (j + 1) * HC]])

    in_maps = []
    for j in range(N_CORES):
        hs = slice(j * HC, (j + 1) * HC)
        vs = slice(j * VC, (j + 1) * VC)
        in_maps.append({
            "ids2": ids2,
            "emb_sh": _ct(emb[:, hs]),
            "attn_WT": attn_WT,
            "attn_b_row": attn_b_row,
            "enc": enc_c,
            "comb_WT_sh": _ct(comb_W[hs, :].T),     # [2048, 128]
            "comb_b_row": _ct(comb_b[hs].reshape(1, HC)),
            "W_ihT_sh": _ct(gate_rows(W_ih, j).T),  # [1024, 384]
            "W_hhT_sh": _ct(gate_rows(W_hh, j).T),
            "h_pm": h_pm,
            "h_chunk_row": _ct(h[hs].reshape(1, HC)),
            "b_ih_row": _ct(gate_rows(b_ih[:, None], j).reshape(1, 3 * HC)),
            "b_hh_row": _ct(gate_rows(b_hh[:, None], j).reshape(1, 3 * HC)),
            "out_WT_sh": np.ascontiguousarray(
                out_W_pad[vs, :].T.astype(ml_dtypes.bfloat16)),  # [1024, 6656] bf16
            "out_b_row": _ct(out_b_pad[vs].reshape(1, VC)),
        })

        trace = bool(int(os.environ.get("KERNEL_TRACE", "0")))
    res = run_bass_kernel_spmd(nc, in_maps, core_ids=list(range(N_CORES)),
                               trace=trace)
    kernel.last_result = res

    logp = np.concatenate([res.results[j]["out_logp"][0] for j in range(N_CORES)])
    log_probs = logp[:V][None, :]
    h_new = res.results[0]["out_h"].reshape(1, 1, H)
    attn_weights = res.results[0]["out_attnw"].reshape(1, L)
    return log_probs, h_new, attn_weights
